# revision 50
# baseline (speedup 1.0000x reference)
"""Trainium2 Bass kernel for nn_CSATransformer_25778393710760.

Math: the reference module (eval mode) computes
    p   = softmax(wt(w1(x) + w2(c) + bsa), dim=-2);  h = x * p
    A   = softmax(mask_diag(sigmoid(si + sj^T)), -1); colsum = A.sum(1)
    ui  = x * colsum[..., None]
    y   = PFF(ui) + ui;  out = LN(y) * g + b
With the given parameters (all biases zero, ln identity), PFF is positively
homogeneous (relu(c*z) = c*relu(z) for c > 0) and colsum > 0, so
    y = diag(colsum) @ (x + PFF(x))
and LayerNorm cancels the positive per-row scale up to the eps term
(relative effect ~ eps/var * (1 - 1/colsum^2) ~ 1e-8).  Hence
    out = LN(relu(x @ pfn_w1) @ pfn_w2 + x) * ln_g + ln_b
to well below f32 noise (verified 4.5e-6 max rel err vs the f32 reference,
identical to the reference's own f32-vs-f64 noise floor).

Sharding: pure data parallel over batch B=8 across the 8 NeuronCores.

Kernel layout per core (one batch example, L=4096 rows of D=128):
8 slabs of 512 rows, fully streaming:
  DMA in -> PE transpose to (d,l) -> w1 matmul + relu -> w2 matmul +
  residual add -> PE transpose back -> bn_stats/bn_aggr LN stats ->
  normalize (DVE/ACT split) -> DMA out.
DMA placement matters: slab-0 per-chunk on the two HWDGE rings, bulk
loads throttled (pool bufs) on the gpsimd SWDGE ring so they do not
steal SDMA bandwidth/queue service from the pipeline-filling loads.
"""

import os
import numpy as np

B, L, DX = 8, 4096, 128
_SLABS = 8          # 512-row slabs per core
_CPS = 4            # 128-row chunks per slab

_prog_cache = {}


def _build_program():
    import concourse.tile as tile
    from concourse import bacc, mybir
    from concourse.bass import ts

    f32 = mybir.dt.float32
    AF = mybir.ActivationFunctionType
    OP = mybir.AluOpType

    nc = bacc.Bacc(None, target_bir_lowering=False)
    bf16 = mybir.dt.bfloat16
    x = nc.dram_tensor("x", [L, DX], f32, kind="ExternalInput")
    w1 = nc.dram_tensor("w1", [DX, DX], bf16, kind="ExternalInput")
    w2 = nc.dram_tensor("w2", [DX, DX], bf16, kind="ExternalInput")
    identp = nc.dram_tensor("identp", [DX, DX + 1], f32, kind="ExternalInput")
    y = nc.dram_tensor("y", [L, DX], f32, kind="ExternalOutput")

    with tile.TileContext(nc) as tc:
        with (
            tc.tile_pool(name="consts", bufs=1) as consts,
            tc.tile_pool(name="io", bufs=4) as io,
            tc.tile_pool(name="work", bufs=4) as work,
            tc.tile_pool(name="small", bufs=4) as small,
            tc.tile_pool(name="xg_pool", bufs=6) as xg_pool,
            tc.tile_pool(name="ps_t", bufs=2, space="PSUM") as ps_t,
            tc.tile_pool(name="ps_mm", bufs=3, space="PSUM") as ps_mm,
            tc.tile_pool(name="ps_out", bufs=2, space="PSUM") as ps_out,
            tc.tile_pool(name="ps_w", bufs=1, space="PSUM") as ps_w,
        ):
            # ---- tiny const DMAs first: transposes gate on ident ----
            identp_sb = consts.tile([128, 129], f32)
            nc.sync.dma_start(out=identp_sb, in_=identp[:, :])
            ident = identp_sb[:, 0:128]
            # bf16 identity for the bf16 transpose-back path
            identb = consts.tile([128, 128], bf16)
            nc.scalar.copy(out=identb, in_=ident)

            # ---- issue all x loads up front so slab 0 lands ASAP ----
            # chunk c = rows congruent to c (mod 4): each partition's bytes
            # are contiguous in HBM (2KB descriptors, ~3x faster loads).
            # LayerNorm is row-wise so the permutation flows through.
            # all x loads on the SWDGE ring with f32->bf16 cast (HWDGE cannot
            # cast); bf16 x makes the transpose-in single-pass and is
            # numerically free since the matmul rhs was already bf16
            xgs = []
            w1_sb = consts.tile([128, 128], bf16)
            w2_sb = consts.tile([128, 128], bf16)
            nc.sync.dma_start(out=w1_sb, in_=w1[:, :])
            nc.scalar.dma_start(out=w2_sb, in_=w2[:, :])
            for g in range(_SLABS):
                xg = xg_pool.tile([128, _CPS, 128], bf16, tag="xg")
                src = x[ts(g, 512), :].rearrange("(p c) d -> p c d", c=_CPS)
                nc.gpsimd.dma_start(out=xg, in_=src)
                xgs.append(xg)
            w1_mm, w2_mm = w1_sb, w2_sb
            eps = consts.tile([128, 1], f32)
            nc.vector.memset(eps, 1e-6)
            # spin the PE on dummy transposes while waiting for x DMAs:
            # ~4us of sustained activity flips the HAM clock gate to 2.4GHz
            # before the real matmuls start (cold fp32 matmuls run at half
            # rate)
            pewarm = ps_w.tile([128, 128], bf16, tag="warm")
            for _ in range(18):
                nc.tensor.transpose(pewarm, identb, identb)
            warmsink = consts.tile([128, 1], bf16)
            nc.vector.tensor_copy(out=warmsink, in_=pewarm[:, 0:1])
            # warm up the ACT table sets off the critical path
            warm = consts.tile([128, 1], f32)
            nc.scalar.activation(out=warm, in_=eps, func=AF.Relu)
            nc.scalar.activation(out=warm, in_=eps, func=AF.Sqrt)
            nc.scalar.activation(out=warm, in_=eps, func=AF.Identity, bias=eps)

            for g in range(_SLABS):
                # ---- transpose to (d, l) layout (all bf16, single-pass) ----
                xtp = ps_t.tile([128, _CPS, 128], bf16, tag="xtp")
                for c in range(_CPS):
                    nc.tensor.transpose(xtp[:, c, :], xgs[g][:, c, :], identb)
                # xT in bf16: single-pass matmuls (fp32 is LOW_HIGH 2-pass)
                xT = work.tile([128, _CPS, 128], bf16, tag="xT")
                nc.scalar.copy(out=xT, in_=xtp)
                xT2 = xT.rearrange("p c d -> p (c d)")

                # ---- PFF: y1T = relu(w1T @ xT); PT = w2T @ y1T + xT ----
                y1p = ps_mm.tile([128, 512], f32, tag="mm")
                nc.tensor.matmul(y1p, lhsT=w1_mm, rhs=xT2, start=True, stop=True)
                y1s = work.tile([128, 512], bf16, tag="y1s")
                nc.scalar.activation(out=y1s, in_=y1p, func=AF.Relu)
                pp = ps_mm.tile([128, 512], f32, tag="mm")
                nc.tensor.matmul(pp, lhsT=w2_mm, rhs=y1s, start=True, stop=True)
                # pt in bf16: transpose-back is single-pass (f32 is 2-pass)
                pt = work.tile([128, 512], bf16, tag="pt")
                nc.vector.tensor_add(out=pt, in0=pp, in1=xT2)

                # keep the HAM busy-window fed so mid-kernel matmuls stay
                # at 2.4GHz (PE duty is ~40%; idle windows re-throttle it)
                nc.tensor.transpose(pewarm, identb, identb)
                nc.tensor.transpose(pewarm, identb, identb)
                nc.tensor.transpose(pewarm, identb, identb)

                # ---- transpose back to (l, d) layout (pn PSUM bf16) ----
                pn = ps_out.tile([128, _CPS, 128], bf16, tag="pn")
                for c in range(_CPS):
                    nc.tensor.transpose(pn[:, c, :], pt[:, ts(c, 128)], identb)

                # ---- LN stats via bn_stats/bn_aggr per chunk ----
                bstats = small.tile([128, _CPS, 6], f32, tag="bstats")
                for c in range(_CPS):
                    nc.vector.bn_stats(out=bstats[:, c, :], in_=pn[:, c, :])
                mv = small.tile([128, _CPS, 2], f32, tag="mv")
                for c in range(_CPS):
                    nc.vector.bn_aggr(out=mv[:, c, :], in_=bstats[:, c, :])

                # rstd = 1/sqrt(var + eps); nmr = -mean * rstd -- one op per
                # slab (per-half splitting doubles the ~200ns fixed costs)
                std = small.tile([128, _CPS], f32, tag="std")
                rstd = small.tile([128, _CPS], f32, tag="rstd")
                nmr = small.tile([128, _CPS], f32, tag="nmr")
                nc.scalar.activation(
                    out=std, in_=mv[:, :, 1], func=AF.Sqrt, scale=1.0, bias=eps
                )
                nc.vector.reciprocal(out=rstd, in_=std)
                nc.vector.scalar_tensor_tensor(
                    out=nmr, in0=mv[:, :, 0], scalar=-1.0, in1=rstd,
                    op0=OP.mult, op1=OP.mult,
                )

                # ---- apply LN from PSUM: out = pn * rstd + nmr ----
                og = io.tile([128, _CPS, 128], f32, tag="og")
                for c in range(_CPS):
                    if c % 2 == 0:
                        nc.vector.tensor_scalar(
                            out=og[:, c, :], in0=pn[:, c, :],
                            scalar1=rstd[:, c : c + 1], scalar2=nmr[:, c : c + 1],
                            op0=OP.mult, op1=OP.add,
                        )
                    else:
                        nc.scalar.activation(
                            out=og[:, c, :], in_=pn[:, c, :], func=AF.Identity,
                            bias=nmr[:, c : c + 1], scale=rstd[:, c : c + 1],
                        )

                dst = y[ts(g, 512), :].rearrange("(p c) d -> p c d", c=_CPS)
                nc.sync.dma_start(out=dst, in_=og)
    nc.finalize()
    return nc


def _ensure_ntff_hook():
    """Register the axon NTFF profiling hook if the image lacks antenv.axon_hooks."""
    try:
        from antenv.axon_hooks import get_axon_ntff_profile_hook  # noqa: F401
        return
    except ImportError:
        pass
    import sys
    import types

    import antenv
    from trn_agent_boot.trn_boot import _ntff_profile_via_ctypes

    hook = _ntff_profile_via_ctypes("/opt/axon/libaxon_pjrt.so")
    mod = types.ModuleType("antenv.axon_hooks")
    mod._hook = hook
    mod.set_axon_ntff_profile_hook = lambda h: setattr(mod, "_hook", h)
    mod.get_axon_ntff_profile_hook = lambda: mod._hook
    sys.modules["antenv.axon_hooks"] = mod
    antenv.axon_hooks = mod


def _run_device(x, w1, w2, trace=False):
    import concourse.bass_utils as bass_utils
    from concourse.bass_utils import run_bass_kernel_spmd

    if trace:
        try:
            _ensure_ntff_hook()
            bass_utils.upload_artifacts = lambda tmpdir: str(tmpdir)
        except Exception as e:  # profiling is best-effort
            print(f"ntff hook unavailable ({e}); running without trace")
            trace = False

    import ml_dtypes

    if "prog" not in _prog_cache:
        _prog_cache["prog"] = _build_program()
    nc = _prog_cache["prog"]
    bf = ml_dtypes.bfloat16
    w1c = np.ascontiguousarray(w1, dtype=np.float32).astype(bf)
    w2c = np.ascontiguousarray(w2, dtype=np.float32).astype(bf)
    identp = np.concatenate(
        [np.eye(DX, dtype=np.float32), np.ones((DX, 1), np.float32)], axis=1
    )
    in_maps = [
        {
            "x": np.ascontiguousarray(x[b], dtype=np.float32),
            "w1": w1c,
            "w2": w2c,
            "identp": identp,
        }
        for b in range(B)
    ]
    res = run_bass_kernel_spmd(
        nc, in_maps, core_ids=list(range(B)), trace=trace,
        trace_cores=list(range(B)) if trace else None,
    )
    kernel.last_result = res
    kernel.last_exec_time_ns = res.exec_time_ns
    return np.stack([r["y"] for r in res.results], axis=0)


def _numpy_fallback(inputs):
    """Faithful (but slow) mirror of the reference for unexpected inputs."""
    f32 = np.float32
    x = np.asarray(inputs["x"], f32)
    c = np.asarray(inputs["c"], f32)
    W1 = np.asarray(inputs["W1"], f32); W2 = np.asarray(inputs["W2"], f32)
    wt_w = np.asarray(inputs["wt_w"], f32); bsa = np.asarray(inputs["bsa"], f32)
    Wsa1 = np.asarray(inputs["Wsa1"], f32); Wsa2 = np.asarray(inputs["Wsa2"], f32)
    wsat_w = np.asarray(inputs["wsat_w"], f32)
    wsat_b = np.asarray(inputs["wsat_b"], f32); bsa1 = np.asarray(inputs["bsa1"], f32)
    pfn_w1 = np.asarray(inputs["pfn_w1"], f32); pfn_b1 = np.asarray(inputs["pfn_b1"], f32)
    pfn_w2 = np.asarray(inputs["pfn_w2"], f32); pfn_b2 = np.asarray(inputs["pfn_b2"], f32)
    ln_g = np.asarray(inputs["ln_g"], f32); ln_b = np.asarray(inputs["ln_b"], f32)
    Bs, Ls, _ = x.shape
    wx = x @ W1
    wq = c @ W2
    logits = (wx + wq[:, None, :] + bsa) @ wt_w
    m = logits.max(-1, keepdims=True)
    e = np.exp(logits - m)
    p = (e / e.sum(-1, keepdims=True))[..., None]
    h = x * p
    si = (h @ Wsa1) @ wsat_w
    sj = (h @ Wsa2) @ wsat_w
    const = bsa1 @ wsat_w + wsat_b
    colsum = np.zeros((Bs, Ls), f32)
    blk = 512
    for b in range(Bs):
        for i0 in range(0, Ls, blk):
            s = 1.0 / (1.0 + np.exp(-(si[b, i0 : i0 + blk, None] + sj[b, None, :] + const)))
            for r in range(s.shape[0]):
                s[r, i0 + r] = -np.inf
            sm = s.max(-1, keepdims=True)
            ee = np.exp(s - sm)
            colsum[b] += (ee / ee.sum(-1, keepdims=True)).sum(0)
    ui = x * colsum[..., None]
    yv = np.maximum(ui @ pfn_w1 + pfn_b1, 0.0)
    yv = yv @ pfn_w2 + pfn_b2 + ui
    mu = yv.mean(-1, keepdims=True)
    var = ((yv - mu) ** 2).mean(-1, keepdims=True)
    return ((yv - mu) / np.sqrt(var + 1e-6) * ln_g + ln_b).astype(f32)


def kernel(**inputs):
    x = np.asarray(inputs["x"], dtype=np.float32)
    pfn_w1 = np.asarray(inputs["pfn_w1"], dtype=np.float32)
    pfn_w2 = np.asarray(inputs["pfn_w2"], dtype=np.float32)

    fast_ok = (
        x.shape == (B, L, DX)
        and not np.any(np.asarray(inputs["pfn_b1"]))
        and not np.any(np.asarray(inputs["pfn_b2"]))
        and np.all(np.asarray(inputs["ln_g"]) == 1.0)
        and not np.any(np.asarray(inputs["ln_b"]))
    )
    if not fast_ok:
        return _numpy_fallback(inputs)

    trace = bool(int(os.environ.get("CSA_TRACE", "0")))
    return _run_device(x, pfn_w1, pfn_w2, trace=trace)


kernel.last_exec_time_ns = None
kernel.last_result = None


# revision 54
# speedup vs baseline: 1.0879x; 1.0879x over previous
"""Trainium2 Bass kernel for nn_CSATransformer_25778393710760.

Math: the reference module (eval mode) computes
    p   = softmax(wt(w1(x) + w2(c) + bsa), dim=-2);  h = x * p
    A   = softmax(mask_diag(sigmoid(si + sj^T)), -1); colsum = A.sum(1)
    ui  = x * colsum[..., None]
    y   = PFF(ui) + ui;  out = LN(y) * g + b
With the given parameters (all biases zero, ln identity), PFF is positively
homogeneous (relu(c*z) = c*relu(z) for c > 0) and colsum > 0, so
    y = diag(colsum) @ (x + PFF(x))
and LayerNorm cancels the positive per-row scale up to the eps term
(relative effect ~ eps/var * (1 - 1/colsum^2) ~ 1e-8).  Hence
    out = LN(relu(x @ pfn_w1) @ pfn_w2 + x) * ln_g + ln_b
to well below f32 noise (verified 4.5e-6 max rel err vs the f32 reference,
identical to the reference's own f32-vs-f64 noise floor).

Sharding: pure data parallel over batch B=8 across the 8 NeuronCores.

Kernel layout per core (one batch example, L=4096 rows of D=128):
8 slabs of 512 rows, fully streaming:
  DMA in -> PE transpose to (d,l) -> w1 matmul + relu -> w2 matmul +
  residual add -> PE transpose back -> bn_stats/bn_aggr LN stats ->
  normalize (DVE/ACT split) -> DMA out.
DMA placement matters: slab-0 per-chunk on the two HWDGE rings, bulk
loads throttled (pool bufs) on the gpsimd SWDGE ring so they do not
steal SDMA bandwidth/queue service from the pipeline-filling loads.
"""

import os
import numpy as np

B, L, DX = 8, 4096, 128
_SLABS = 8          # 512-row slabs per core
_CPS = 4            # 128-row chunks per slab

_prog_cache = {}


def _build_program():
    import concourse.tile as tile
    from concourse import bacc, mybir
    from concourse.bass import ts

    f32 = mybir.dt.float32
    AF = mybir.ActivationFunctionType
    OP = mybir.AluOpType

    nc = bacc.Bacc(None, target_bir_lowering=False)
    bf16 = mybir.dt.bfloat16
    x = nc.dram_tensor("x", [L, DX], f32, kind="ExternalInput")
    w1 = nc.dram_tensor("w1", [DX, DX], bf16, kind="ExternalInput")
    w2 = nc.dram_tensor("w2", [DX, DX], bf16, kind="ExternalInput")
    identp = nc.dram_tensor("identp", [DX, DX + 1], f32, kind="ExternalInput")
    y = nc.dram_tensor("y", [L, DX], f32, kind="ExternalOutput")

    with tile.TileContext(nc) as tc:
        with (
            tc.tile_pool(name="consts", bufs=1) as consts,
            tc.tile_pool(name="io", bufs=3) as io,
            tc.tile_pool(name="work", bufs=3) as work,
            tc.tile_pool(name="small", bufs=4) as small,
            tc.tile_pool(name="xg_pool", bufs=6) as xg_pool,
            tc.tile_pool(name="ps_t", bufs=2, space="PSUM") as ps_t,
            tc.tile_pool(name="ps_mm", bufs=3, space="PSUM") as ps_mm,
            tc.tile_pool(name="ps_out", bufs=3, space="PSUM") as ps_out,
        ):
            # ---- tiny const DMAs first: transposes gate on ident ----
            identp_sb = consts.tile([128, 129], f32)
            nc.sync.dma_start(out=identp_sb, in_=identp[:, :])
            ident = identp_sb[:, 0:128]
            # bf16 identity for the bf16 transpose-back path
            identb = consts.tile([128, 128], bf16)
            nc.scalar.copy(out=identb, in_=ident)

            # ---- issue all x loads up front so slab 0 lands ASAP ----
            # chunk c = rows congruent to c (mod 4): each partition's bytes
            # are contiguous in HBM (2KB descriptors, ~3x faster loads).
            # LayerNorm is row-wise so the permutation flows through.
            # all x loads on the SWDGE ring with f32->bf16 cast (HWDGE cannot
            # cast); bf16 x makes the transpose-in single-pass and is
            # numerically free since the matmul rhs was already bf16
            xgs = []
            w1_sb = consts.tile([128, 128], bf16)
            w2_sb = consts.tile([128, 128], bf16)
            nc.sync.dma_start(out=w1_sb, in_=w1[:, :])
            nc.scalar.dma_start(out=w2_sb, in_=w2[:, :])
            for g in range(_SLABS):
                xg = xg_pool.tile([128, _CPS, 128], bf16, tag="xg")
                src = x[ts(g, 512), :].rearrange("(p c) d -> p c d", c=_CPS)
                nc.gpsimd.dma_start(out=xg, in_=src)
                xgs.append(xg)
            w1_mm, w2_mm = w1_sb, w2_sb
            eps = consts.tile([128, 1], f32)
            nc.vector.memset(eps, 1e-6)
            # spin the PE on dummy transposes while waiting for x DMAs:
            # ~4us of sustained activity flips the HAM clock gate to 2.4GHz
            # before the real matmuls start (cold fp32 matmuls run at half
            # rate)
            pewarm = ps_t.tile([128, _CPS, 128], bf16, tag="xtp")
            for _ in range(18):
                nc.tensor.transpose(pewarm[:, 0, :], identb, identb)
            warmsink = consts.tile([128, 1], bf16)
            nc.vector.tensor_copy(out=warmsink, in_=pewarm[:, 0, 0:1])
            # warm up the ACT table sets off the critical path
            warm = consts.tile([128, 1], f32)
            nc.scalar.activation(out=warm, in_=eps, func=AF.Relu)
            nc.scalar.activation(out=warm, in_=eps, func=AF.Sqrt)
            nc.scalar.activation(out=warm, in_=eps, func=AF.Identity, bias=eps)

            for g in range(_SLABS):
                # ---- transpose to (d, l) layout (all bf16, single-pass) ----
                xtp = ps_t.tile([128, _CPS, 128], bf16, tag="xtp")
                for c in range(_CPS):
                    nc.tensor.transpose(xtp[:, c, :], xgs[g][:, c, :], identb)
                # xT in bf16: single-pass matmuls (fp32 is LOW_HIGH 2-pass)
                xT = work.tile([128, _CPS, 128], bf16, tag="xT")
                nc.scalar.copy(out=xT, in_=xtp)
                xT2 = xT.rearrange("p c d -> p (c d)")

                # ---- PFF: y1T = relu(w1T @ xT); PT = w2T @ y1T + xT ----
                y1p = ps_mm.tile([128, 512], f32, tag="mm")
                nc.tensor.matmul(y1p, lhsT=w1_mm, rhs=xT2, start=True, stop=True)
                y1s = work.tile([128, 512], bf16, tag="y1s")
                nc.scalar.activation(out=y1s, in_=y1p, func=AF.Relu)
                pp = ps_mm.tile([128, 512], f32, tag="mm")
                nc.tensor.matmul(pp, lhsT=w2_mm, rhs=y1s, start=True, stop=True)
                # pt in bf16: transpose-back is single-pass (f32 is 2-pass)
                pt = work.tile([128, 512], bf16, tag="pt")
                nc.vector.tensor_add(out=pt, in0=pp, in1=xT2)

                # keep the HAM busy-window fed so mid-kernel matmuls stay at
                # 2.4GHz; targets this slab's xtp, already consumed by the
                # xT copy above, so no extra PSUM bank is needed
                nc.tensor.transpose(xtp[:, 0, :], identb, identb)
                nc.tensor.transpose(xtp[:, 0, :], identb, identb)
                nc.tensor.transpose(xtp[:, 0, :], identb, identb)

                # ---- transpose back to (l, d) layout (pn PSUM bf16) ----
                pn = ps_out.tile([128, _CPS, 128], bf16, tag="pn")
                for c in range(_CPS):
                    nc.tensor.transpose(pn[:, c, :], pt[:, ts(c, 128)], identb)

                # ---- LN stats via bn_stats/bn_aggr per chunk ----
                bstats = small.tile([128, _CPS, 6], f32, tag="bstats")
                for c in range(_CPS):
                    nc.vector.bn_stats(out=bstats[:, c, :], in_=pn[:, c, :])
                mv = small.tile([128, _CPS, 2], f32, tag="mv")
                for c in range(_CPS):
                    nc.vector.bn_aggr(out=mv[:, c, :], in_=bstats[:, c, :])

                # rstd = 1/sqrt(var + eps); nmr = -mean * rstd -- one op per
                # slab (per-half splitting doubles the ~200ns fixed costs)
                std = small.tile([128, _CPS], f32, tag="std")
                rstd = small.tile([128, _CPS], f32, tag="rstd")
                nmr = small.tile([128, _CPS], f32, tag="nmr")
                nc.scalar.activation(
                    out=std, in_=mv[:, :, 1], func=AF.Sqrt, scale=1.0, bias=eps
                )
                nc.vector.reciprocal(out=rstd, in_=std)
                nc.vector.scalar_tensor_tensor(
                    out=nmr, in0=mv[:, :, 0], scalar=-1.0, in1=rstd,
                    op0=OP.mult, op1=OP.mult,
                )

                # ---- apply LN from PSUM: out = pn * rstd + nmr ----
                og = io.tile([128, _CPS, 128], f32, tag="og")
                for c in range(_CPS):
                    if c % 2 == 0:
                        nc.vector.tensor_scalar(
                            out=og[:, c, :], in0=pn[:, c, :],
                            scalar1=rstd[:, c : c + 1], scalar2=nmr[:, c : c + 1],
                            op0=OP.mult, op1=OP.add,
                        )
                    else:
                        nc.scalar.activation(
                            out=og[:, c, :], in_=pn[:, c, :], func=AF.Identity,
                            bias=nmr[:, c : c + 1], scale=rstd[:, c : c + 1],
                        )

                dst = y[ts(g, 512), :].rearrange("(p c) d -> p c d", c=_CPS)
                nc.sync.dma_start(out=dst, in_=og)
    nc.finalize()
    return nc


def _ensure_ntff_hook():
    """Register the axon NTFF profiling hook if the image lacks antenv.axon_hooks."""
    try:
        from antenv.axon_hooks import get_axon_ntff_profile_hook  # noqa: F401
        return
    except ImportError:
        pass
    import sys
    import types

    import antenv
    from trn_agent_boot.trn_boot import _ntff_profile_via_ctypes

    hook = _ntff_profile_via_ctypes("/opt/axon/libaxon_pjrt.so")
    mod = types.ModuleType("antenv.axon_hooks")
    mod._hook = hook
    mod.set_axon_ntff_profile_hook = lambda h: setattr(mod, "_hook", h)
    mod.get_axon_ntff_profile_hook = lambda: mod._hook
    sys.modules["antenv.axon_hooks"] = mod
    antenv.axon_hooks = mod


def _run_device(x, w1, w2, trace=False):
    import concourse.bass_utils as bass_utils
    from concourse.bass_utils import run_bass_kernel_spmd

    if trace:
        try:
            _ensure_ntff_hook()
            bass_utils.upload_artifacts = lambda tmpdir: str(tmpdir)
        except Exception as e:  # profiling is best-effort
            print(f"ntff hook unavailable ({e}); running without trace")
            trace = False

    import ml_dtypes

    if "prog" not in _prog_cache:
        _prog_cache["prog"] = _build_program()
    nc = _prog_cache["prog"]
    bf = ml_dtypes.bfloat16
    w1c = np.ascontiguousarray(w1, dtype=np.float32).astype(bf)
    w2c = np.ascontiguousarray(w2, dtype=np.float32).astype(bf)
    identp = np.concatenate(
        [np.eye(DX, dtype=np.float32), np.ones((DX, 1), np.float32)], axis=1
    )
    in_maps = [
        {
            "x": np.ascontiguousarray(x[b], dtype=np.float32),
            "w1": w1c,
            "w2": w2c,
            "identp": identp,
        }
        for b in range(B)
    ]
    res = run_bass_kernel_spmd(
        nc, in_maps, core_ids=list(range(B)), trace=trace,
        trace_cores=list(range(B)) if trace else None,
    )
    kernel.last_result = res
    kernel.last_exec_time_ns = res.exec_time_ns
    return np.stack([r["y"] for r in res.results], axis=0)


def _numpy_fallback(inputs):
    """Faithful (but slow) mirror of the reference for unexpected inputs."""
    f32 = np.float32
    x = np.asarray(inputs["x"], f32)
    c = np.asarray(inputs["c"], f32)
    W1 = np.asarray(inputs["W1"], f32); W2 = np.asarray(inputs["W2"], f32)
    wt_w = np.asarray(inputs["wt_w"], f32); bsa = np.asarray(inputs["bsa"], f32)
    Wsa1 = np.asarray(inputs["Wsa1"], f32); Wsa2 = np.asarray(inputs["Wsa2"], f32)
    wsat_w = np.asarray(inputs["wsat_w"], f32)
    wsat_b = np.asarray(inputs["wsat_b"], f32); bsa1 = np.asarray(inputs["bsa1"], f32)
    pfn_w1 = np.asarray(inputs["pfn_w1"], f32); pfn_b1 = np.asarray(inputs["pfn_b1"], f32)
    pfn_w2 = np.asarray(inputs["pfn_w2"], f32); pfn_b2 = np.asarray(inputs["pfn_b2"], f32)
    ln_g = np.asarray(inputs["ln_g"], f32); ln_b = np.asarray(inputs["ln_b"], f32)
    Bs, Ls, _ = x.shape
    wx = x @ W1
    wq = c @ W2
    logits = (wx + wq[:, None, :] + bsa) @ wt_w
    m = logits.max(-1, keepdims=True)
    e = np.exp(logits - m)
    p = (e / e.sum(-1, keepdims=True))[..., None]
    h = x * p
    si = (h @ Wsa1) @ wsat_w
    sj = (h @ Wsa2) @ wsat_w
    const = bsa1 @ wsat_w + wsat_b
    colsum = np.zeros((Bs, Ls), f32)
    blk = 512
    for b in range(Bs):
        for i0 in range(0, Ls, blk):
            s = 1.0 / (1.0 + np.exp(-(si[b, i0 : i0 + blk, None] + sj[b, None, :] + const)))
            for r in range(s.shape[0]):
                s[r, i0 + r] = -np.inf
            sm = s.max(-1, keepdims=True)
            ee = np.exp(s - sm)
            colsum[b] += (ee / ee.sum(-1, keepdims=True)).sum(0)
    ui = x * colsum[..., None]
    yv = np.maximum(ui @ pfn_w1 + pfn_b1, 0.0)
    yv = yv @ pfn_w2 + pfn_b2 + ui
    mu = yv.mean(-1, keepdims=True)
    var = ((yv - mu) ** 2).mean(-1, keepdims=True)
    return ((yv - mu) / np.sqrt(var + 1e-6) * ln_g + ln_b).astype(f32)


def kernel(**inputs):
    x = np.asarray(inputs["x"], dtype=np.float32)
    pfn_w1 = np.asarray(inputs["pfn_w1"], dtype=np.float32)
    pfn_w2 = np.asarray(inputs["pfn_w2"], dtype=np.float32)

    fast_ok = (
        x.shape == (B, L, DX)
        and not np.any(np.asarray(inputs["pfn_b1"]))
        and not np.any(np.asarray(inputs["pfn_b2"]))
        and np.all(np.asarray(inputs["ln_g"]) == 1.0)
        and not np.any(np.asarray(inputs["ln_b"]))
    )
    if not fast_ok:
        return _numpy_fallback(inputs)

    trace = bool(int(os.environ.get("CSA_TRACE", "0")))
    return _run_device(x, pfn_w1, pfn_w2, trace=trace)


kernel.last_exec_time_ns = None
kernel.last_result = None


# revision 58
# speedup vs baseline: 1.0946x; 1.0062x over previous
"""Trainium2 Bass kernel for nn_CSATransformer_25778393710760.

Math: the reference module (eval mode) computes
    p   = softmax(wt(w1(x) + w2(c) + bsa), dim=-2);  h = x * p
    A   = softmax(mask_diag(sigmoid(si + sj^T)), -1); colsum = A.sum(1)
    ui  = x * colsum[..., None]
    y   = PFF(ui) + ui;  out = LN(y) * g + b
With the given parameters (all biases zero, ln identity), PFF is positively
homogeneous (relu(c*z) = c*relu(z) for c > 0) and colsum > 0, so
    y = diag(colsum) @ (x + PFF(x))
and LayerNorm cancels the positive per-row scale up to the eps term
(relative effect ~ eps/var * (1 - 1/colsum^2) ~ 1e-8).  Hence
    out = LN(relu(x @ pfn_w1) @ pfn_w2 + x) * ln_g + ln_b
to well below f32 noise (verified 4.5e-6 max rel err vs the f32 reference,
identical to the reference's own f32-vs-f64 noise floor).

Sharding: pure data parallel over batch B=8 across the 8 NeuronCores.

Kernel layout per core (one batch example, L=4096 rows of D=128):
8 slabs of 512 rows, fully streaming:
  DMA in -> PE transpose to (d,l) -> w1 matmul + relu -> w2 matmul +
  residual add -> PE transpose back -> bn_stats/bn_aggr LN stats ->
  normalize (DVE/ACT split) -> DMA out.
DMA placement matters: slab-0 per-chunk on the two HWDGE rings, bulk
loads throttled (pool bufs) on the gpsimd SWDGE ring so they do not
steal SDMA bandwidth/queue service from the pipeline-filling loads.
"""

import os
import numpy as np

B, L, DX = 8, 4096, 128
_SLABS = 8          # 512-row slabs per core
_CPS = 4            # 128-row chunks per slab

_prog_cache = {}


def _build_program():
    import concourse.tile as tile
    from concourse import bacc, mybir
    from concourse.bass import ts

    f32 = mybir.dt.float32
    AF = mybir.ActivationFunctionType
    OP = mybir.AluOpType

    nc = bacc.Bacc(None, target_bir_lowering=False)
    bf16 = mybir.dt.bfloat16
    x = nc.dram_tensor("x", [L, DX], f32, kind="ExternalInput")
    w1 = nc.dram_tensor("w1", [DX, DX], bf16, kind="ExternalInput")
    w2 = nc.dram_tensor("w2", [DX, DX], bf16, kind="ExternalInput")
    identp = nc.dram_tensor("identp", [DX, DX + 1], f32, kind="ExternalInput")
    y = nc.dram_tensor("y", [L, DX], f32, kind="ExternalOutput")

    with tile.TileContext(nc) as tc:
        with (
            tc.tile_pool(name="consts", bufs=1) as consts,
            tc.tile_pool(name="io", bufs=3) as io,
            tc.tile_pool(name="work", bufs=3) as work,
            tc.tile_pool(name="small", bufs=4) as small,
            tc.tile_pool(name="xg_pool", bufs=6) as xg_pool,
            tc.tile_pool(name="ps_t", bufs=2, space="PSUM") as ps_t,
            tc.tile_pool(name="ps_mm", bufs=3, space="PSUM") as ps_mm,
            tc.tile_pool(name="ps_out", bufs=3, space="PSUM") as ps_out,
        ):
            # ---- tiny const DMAs first: transposes gate on ident ----
            identp_sb = consts.tile([128, 129], f32)
            nc.sync.dma_start(out=identp_sb, in_=identp[:, :])
            ident = identp_sb[:, 0:128]
            # bf16 identity for the bf16 transpose-back path
            identb = consts.tile([128, 128], bf16)
            nc.scalar.copy(out=identb, in_=ident)

            # ---- issue all x loads up front so slab 0 lands ASAP ----
            # chunk c = rows congruent to c (mod 4): each partition's bytes
            # are contiguous in HBM (2KB descriptors, ~3x faster loads).
            # LayerNorm is row-wise so the permutation flows through.
            # all x loads on the SWDGE ring with f32->bf16 cast (HWDGE cannot
            # cast); bf16 x makes the transpose-in single-pass and is
            # numerically free since the matmul rhs was already bf16
            xgs = []
            # slab 0 fast path: plain f32 on the idle sync HWDGE ring (HWDGE
            # cannot cast) so the pipeline starts ~2.5us earlier than the
            # SWDGE cast-load path can deliver
            xg0 = xg_pool.tile([128, _CPS, 128], f32, tag="xg0")
            nc.sync.dma_start(
                out=xg0, in_=x[ts(0, 512), :].rearrange("(p c) d -> p c d", c=_CPS)
            )
            xgs.append(xg0)
            w1_sb = consts.tile([128, 128], bf16)
            w2_sb = consts.tile([128, 128], bf16)
            nc.sync.dma_start(out=w1_sb, in_=w1[:, :])
            nc.scalar.dma_start(out=w2_sb, in_=w2[:, :])
            for g in range(1, _SLABS):
                xg = xg_pool.tile([128, _CPS, 128], bf16, tag="xg")
                src = x[ts(g, 512), :].rearrange("(p c) d -> p c d", c=_CPS)
                nc.gpsimd.dma_start(out=xg, in_=src)
                xgs.append(xg)
            w1_mm, w2_mm = w1_sb, w2_sb
            eps = consts.tile([128, 1], f32)
            nc.vector.memset(eps, 1e-6)
            # spin the PE on dummy transposes while waiting for x DMAs:
            # ~4us of sustained activity flips the HAM clock gate to 2.4GHz
            # before the real matmuls start (cold fp32 matmuls run at half
            # rate)
            pewarm = ps_t.tile([128, _CPS, 128], bf16, tag="xtp")
            for _ in range(18):
                nc.tensor.transpose(pewarm[:, 0, :], identb, identb)
            warmsink = consts.tile([128, 1], bf16)
            nc.vector.tensor_copy(out=warmsink, in_=pewarm[:, 0, 0:1])
            # warm up the ACT table sets off the critical path
            warm = consts.tile([128, 1], f32)
            nc.scalar.activation(out=warm, in_=eps, func=AF.Relu)
            nc.scalar.activation(out=warm, in_=eps, func=AF.Sqrt)
            nc.scalar.activation(out=warm, in_=eps, func=AF.Identity, bias=eps)

            for g in range(_SLABS):
                # ---- transpose to (d, l) layout (bf16 single-pass; slab 0
                # is f32 from the fast-path load and borrows an mm buffer) ----
                if g == 0:
                    xtp = ps_mm.tile([128, 512], f32, tag="mm")
                    for c in range(_CPS):
                        nc.tensor.transpose(
                            xtp[:, ts(c, 128)], xgs[0][:, c, :], ident
                        )
                else:
                    xtp = ps_t.tile([128, _CPS, 128], bf16, tag="xtp")
                    for c in range(_CPS):
                        nc.tensor.transpose(xtp[:, c, :], xgs[g][:, c, :], identb)
                # xT in bf16: single-pass matmuls (fp32 is LOW_HIGH 2-pass)
                xT = work.tile([128, _CPS, 128], bf16, tag="xT")
                xT2 = xT.rearrange("p c d -> p (c d)")
                nc.scalar.copy(out=xT2 if g == 0 else xT, in_=xtp)

                # ---- PFF: y1T = relu(w1T @ xT); PT = w2T @ y1T + xT ----
                y1p = ps_mm.tile([128, 512], f32, tag="mm")
                nc.tensor.matmul(y1p, lhsT=w1_mm, rhs=xT2, start=True, stop=True)
                y1s = work.tile([128, 512], bf16, tag="y1s")
                nc.scalar.activation(out=y1s, in_=y1p, func=AF.Relu)
                pp = ps_mm.tile([128, 512], f32, tag="mm")
                nc.tensor.matmul(pp, lhsT=w2_mm, rhs=y1s, start=True, stop=True)
                # pt in bf16: transpose-back is single-pass (f32 is 2-pass)
                pt = work.tile([128, 512], bf16, tag="pt")
                nc.vector.tensor_add(out=pt, in0=pp, in1=xT2)

                # keep the HAM busy-window fed so mid-kernel matmuls stay at
                # 2.4GHz; targets this slab's xtp, already consumed by the
                # xT copy above, so no extra PSUM bank is needed
                ka = xtp[:, ts(0, 128)] if g == 0 else xtp[:, 0, :]
                ki = ident if g == 0 else identb
                nc.tensor.transpose(ka, ki, ki)
                nc.tensor.transpose(ka, ki, ki)
                nc.tensor.transpose(ka, ki, ki)

                # ---- transpose back to (l, d) layout (pn PSUM bf16) ----
                pn = ps_out.tile([128, _CPS, 128], bf16, tag="pn")
                for c in range(_CPS):
                    nc.tensor.transpose(pn[:, c, :], pt[:, ts(c, 128)], identb)

                # ---- LN stats via bn_stats/bn_aggr per chunk ----
                bstats = small.tile([128, _CPS, 6], f32, tag="bstats")
                for c in range(_CPS):
                    nc.vector.bn_stats(out=bstats[:, c, :], in_=pn[:, c, :])
                mv = small.tile([128, _CPS, 2], f32, tag="mv")
                for c in range(_CPS):
                    nc.vector.bn_aggr(out=mv[:, c, :], in_=bstats[:, c, :])

                # rstd = 1/sqrt(var + eps); nmr = -mean * rstd -- one op per
                # slab (per-half splitting doubles the ~200ns fixed costs)
                std = small.tile([128, _CPS], f32, tag="std")
                rstd = small.tile([128, _CPS], f32, tag="rstd")
                nmr = small.tile([128, _CPS], f32, tag="nmr")
                nc.scalar.activation(
                    out=std, in_=mv[:, :, 1], func=AF.Sqrt, scale=1.0, bias=eps
                )
                nc.vector.reciprocal(out=rstd, in_=std)
                nc.vector.scalar_tensor_tensor(
                    out=nmr, in0=mv[:, :, 0], scalar=-1.0, in1=rstd,
                    op0=OP.mult, op1=OP.mult,
                )

                # ---- apply LN from PSUM: out = pn * rstd + nmr ----
                og = io.tile([128, _CPS, 128], f32, tag="og")
                for c in range(_CPS):
                    if c % 2 == 0:
                        nc.vector.tensor_scalar(
                            out=og[:, c, :], in0=pn[:, c, :],
                            scalar1=rstd[:, c : c + 1], scalar2=nmr[:, c : c + 1],
                            op0=OP.mult, op1=OP.add,
                        )
                    else:
                        nc.scalar.activation(
                            out=og[:, c, :], in_=pn[:, c, :], func=AF.Identity,
                            bias=nmr[:, c : c + 1], scale=rstd[:, c : c + 1],
                        )

                dst = y[ts(g, 512), :].rearrange("(p c) d -> p c d", c=_CPS)
                nc.sync.dma_start(out=dst, in_=og)
    nc.finalize()
    return nc


def _ensure_ntff_hook():
    """Register the axon NTFF profiling hook if the image lacks antenv.axon_hooks."""
    try:
        from antenv.axon_hooks import get_axon_ntff_profile_hook  # noqa: F401
        return
    except ImportError:
        pass
    import sys
    import types

    import antenv
    from trn_agent_boot.trn_boot import _ntff_profile_via_ctypes

    hook = _ntff_profile_via_ctypes("/opt/axon/libaxon_pjrt.so")
    mod = types.ModuleType("antenv.axon_hooks")
    mod._hook = hook
    mod.set_axon_ntff_profile_hook = lambda h: setattr(mod, "_hook", h)
    mod.get_axon_ntff_profile_hook = lambda: mod._hook
    sys.modules["antenv.axon_hooks"] = mod
    antenv.axon_hooks = mod


def _run_device(x, w1, w2, trace=False):
    import concourse.bass_utils as bass_utils
    from concourse.bass_utils import run_bass_kernel_spmd

    if trace:
        try:
            _ensure_ntff_hook()
            bass_utils.upload_artifacts = lambda tmpdir: str(tmpdir)
        except Exception as e:  # profiling is best-effort
            print(f"ntff hook unavailable ({e}); running without trace")
            trace = False

    import ml_dtypes

    if "prog" not in _prog_cache:
        _prog_cache["prog"] = _build_program()
    nc = _prog_cache["prog"]
    bf = ml_dtypes.bfloat16
    w1c = np.ascontiguousarray(w1, dtype=np.float32).astype(bf)
    w2c = np.ascontiguousarray(w2, dtype=np.float32).astype(bf)
    identp = np.concatenate(
        [np.eye(DX, dtype=np.float32), np.ones((DX, 1), np.float32)], axis=1
    )
    in_maps = [
        {
            "x": np.ascontiguousarray(x[b], dtype=np.float32),
            "w1": w1c,
            "w2": w2c,
            "identp": identp,
        }
        for b in range(B)
    ]
    res = run_bass_kernel_spmd(
        nc, in_maps, core_ids=list(range(B)), trace=trace,
        trace_cores=list(range(B)) if trace else None,
    )
    kernel.last_result = res
    kernel.last_exec_time_ns = res.exec_time_ns
    return np.stack([r["y"] for r in res.results], axis=0)


def _numpy_fallback(inputs):
    """Faithful (but slow) mirror of the reference for unexpected inputs."""
    f32 = np.float32
    x = np.asarray(inputs["x"], f32)
    c = np.asarray(inputs["c"], f32)
    W1 = np.asarray(inputs["W1"], f32); W2 = np.asarray(inputs["W2"], f32)
    wt_w = np.asarray(inputs["wt_w"], f32); bsa = np.asarray(inputs["bsa"], f32)
    Wsa1 = np.asarray(inputs["Wsa1"], f32); Wsa2 = np.asarray(inputs["Wsa2"], f32)
    wsat_w = np.asarray(inputs["wsat_w"], f32)
    wsat_b = np.asarray(inputs["wsat_b"], f32); bsa1 = np.asarray(inputs["bsa1"], f32)
    pfn_w1 = np.asarray(inputs["pfn_w1"], f32); pfn_b1 = np.asarray(inputs["pfn_b1"], f32)
    pfn_w2 = np.asarray(inputs["pfn_w2"], f32); pfn_b2 = np.asarray(inputs["pfn_b2"], f32)
    ln_g = np.asarray(inputs["ln_g"], f32); ln_b = np.asarray(inputs["ln_b"], f32)
    Bs, Ls, _ = x.shape
    wx = x @ W1
    wq = c @ W2
    logits = (wx + wq[:, None, :] + bsa) @ wt_w
    m = logits.max(-1, keepdims=True)
    e = np.exp(logits - m)
    p = (e / e.sum(-1, keepdims=True))[..., None]
    h = x * p
    si = (h @ Wsa1) @ wsat_w
    sj = (h @ Wsa2) @ wsat_w
    const = bsa1 @ wsat_w + wsat_b
    colsum = np.zeros((Bs, Ls), f32)
    blk = 512
    for b in range(Bs):
        for i0 in range(0, Ls, blk):
            s = 1.0 / (1.0 + np.exp(-(si[b, i0 : i0 + blk, None] + sj[b, None, :] + const)))
            for r in range(s.shape[0]):
                s[r, i0 + r] = -np.inf
            sm = s.max(-1, keepdims=True)
            ee = np.exp(s - sm)
            colsum[b] += (ee / ee.sum(-1, keepdims=True)).sum(0)
    ui = x * colsum[..., None]
    yv = np.maximum(ui @ pfn_w1 + pfn_b1, 0.0)
    yv = yv @ pfn_w2 + pfn_b2 + ui
    mu = yv.mean(-1, keepdims=True)
    var = ((yv - mu) ** 2).mean(-1, keepdims=True)
    return ((yv - mu) / np.sqrt(var + 1e-6) * ln_g + ln_b).astype(f32)


def kernel(**inputs):
    x = np.asarray(inputs["x"], dtype=np.float32)
    pfn_w1 = np.asarray(inputs["pfn_w1"], dtype=np.float32)
    pfn_w2 = np.asarray(inputs["pfn_w2"], dtype=np.float32)

    fast_ok = (
        x.shape == (B, L, DX)
        and not np.any(np.asarray(inputs["pfn_b1"]))
        and not np.any(np.asarray(inputs["pfn_b2"]))
        and np.all(np.asarray(inputs["ln_g"]) == 1.0)
        and not np.any(np.asarray(inputs["ln_b"]))
    )
    if not fast_ok:
        return _numpy_fallback(inputs)

    trace = bool(int(os.environ.get("CSA_TRACE", "0")))
    return _run_device(x, pfn_w1, pfn_w2, trace=trace)


kernel.last_exec_time_ns = None
kernel.last_result = None
